# revision 49
# baseline (speedup 1.0000x reference)
"""Trainium2 Bass kernel for nn_Long_term_atention.

Reference structure: scores for every query row are identical (the torch code
broadcasts a single (B,1,K) score row), so softmax(QK^T masked) @ V' reduces to
a causal *prefix softmax*:
    unmasked row q:  out_att[q] = (sum_{k<=q} a_kq V_k) @ W_v
    masked row q:    out_att[q] = (sum_all V_k) @ W_v / K_LEN
with a_kq = w_k / Z_q (normalized attention weights, folded on host),
w_k = exp(s_k - max s), Z_q = cumsum(w)[q], s = K @ (W_k (W_q^T Q)) / temp.

Host precomputes all O(B*K) quantities (s, w, Z, per-128-block partial sums S,
totals T, mask folding, the 1/Z column scaling).  The device computes, per
batch:
  P^T[d, q] = sum_{k<=q} a_kq V[k, d]    (block-triangular bf16 matmuls)
  V_att     = (P^T)^T @ W_v              (bf16 matmuls, PSUM f32)
  x         = V_att + V                  (DVE scalar_tensor_tensor, accum=sum)
  sum x^2   = ACT Square accum
  out       = (x - mu) * rsqrt(var+eps)  (DVE tensor_scalar, bf16 out)
Sharding: data-parallel over batch, 2 batches per core on 8 cores.
I/O in bf16 (V in, out); host does the final bf16->f32 upcast.
"""

import os
import sys

import numpy as np

sys.path.insert(0, "/opt/trn_rl_repo")

B, K_LEN, D = 16, 2048, 512
N_CORES = 8
BPC = B // N_CORES          # batches per core
NKB = K_LEN // 128          # 16 k-blocks of 128
NQC = K_LEN // 512          # 4 q-chunks of 512
TEMP_EPS = 1e-06
LN_EPS = 1e-05

# 'f32r' (full fp32-ish precision, full rate at N>=256) or 'bf16'
MM_MODE = os.environ.get("BASS_MM_MODE", "bf16")
# which dc-indices of the P^T evacuation run on DVE (rest on ACT)
EVAC_DVE = tuple(int(c) for c in os.environ.get("BASS_EVAC_DVE", "23") if c.isdigit())
# dummy matmuls issued before the first real work to keep the PE warm
PREWARM_MMS = int(os.environ.get("BASS_PREWARM", "0"))
# dummy matmuls between batch-0 chunks (PE is DMA-gated there; keeps HAM warm)
PREWARM_GAP = int(os.environ.get("BASS_PREWARM_GAP", "0"))

_COMPILED = {}


def _host_prep(Q, K, V, mask, W_q, W_k, W_v):
    """All O(B*K_LEN) precompute, float64 for stability."""
    import ml_dtypes
    bf16 = ml_dtypes.bfloat16
    Qd = Q.astype(np.float64)
    Kd = K.astype(np.float64)
    Vd = V.astype(np.float64)
    m_f = mask.astype(np.float64)           # (B, K) 1.0 where masked
    temp = np.sqrt(np.float64(D)) + TEMP_EPS

    a_t = (Qd @ W_q.astype(np.float64)) @ W_k.astype(np.float64).T / temp  # (B, D)
    s = np.einsum("bkd,bd->bk", Kd, a_t)                                   # (B, K)
    w = np.exp(s - s.max(axis=1, keepdims=True))                           # (B, K)
    Z = np.cumsum(w, axis=1)
    Zp = np.where(mask, np.float64(K_LEN), Z)
    invz = 1.0 / Zp                                                        # (B, K)

    wg = w.reshape(B, NKB, 128)                                            # (B,16,128)
    mg = m_f.reshape(B, NKB, 128)                                          # (B,16,128)
    izg = invz.reshape(B, NKB, 128)
    kl = np.arange(128)[:, None]
    ql = np.arange(128)[None, :]
    tri = (kl <= ql).astype(np.float64)                                    # (128,128)
    # A_diag[b, kl, 128*j + ql] = w[b,128j+kl] * (kl<=ql) * (1-m) * invz [at ql]
    adiag = (wg[:, :, :, None] * tri[None, None]
             * ((1.0 - mg) * izg)[:, :, None, :])
    adiag = np.ascontiguousarray(
        adiag.transpose(0, 2, 1, 3).reshape(B, 128, K_LEN))

    # S[b,i,d] = sum_{k in block i} w V ;  T[b,d] = sum_k V
    Sb = np.einsum("bik,bikd->bid", wg, Vd.reshape(B, NKB, 128, D))        # (B,16,D)
    Tv = Vd.sum(axis=1)                                                    # (B,D)
    s_aug = np.concatenate([Sb, Tv[:, None, :]], axis=1)                   # (B,17,D)

    # cm[b,i,q] = (i < q//128) * (1 - m) * invz ; row 16 = m * invz (= m/K)
    qblk = (np.arange(K_LEN) // 128)[None, None, :]
    iidx = np.arange(NKB)[None, :, None]
    cm = (iidx < qblk).astype(np.float64) * ((1.0 - m_f) * invz)[:, None, :]
    cm_aug = np.concatenate([cm, (m_f * invz)[:, None, :]], axis=1)        # (B,17,K)

    # V in bf16, [b, p, j, d] = V[b, 128j+p, d]
    vb = np.ascontiguousarray(
        Vd.reshape(B, NKB, 128, D).transpose(0, 2, 1, 3)).astype(bf16)

    scm = np.ascontiguousarray(np.concatenate([s_aug, cm_aug], axis=2))
    return dict(
        adiag=adiag.astype(bf16),
        scm=scm.astype(bf16 if MM_MODE == "bf16" else np.float32),
        vb=vb,
    )


def _patch_drain_split(tile, mybir):
    """Tile's kernel-tail drain carries one wait per semaphore lane on a
    single Drain instruction; walrus allows only one wait per instruction.
    Split the waits over a chain of drains."""
    if getattr(tile.TileContext, "_drain_split_patched", False):
        return
    from concourse.vector_clock import ScopedClock

    def _drain_and_barrier(self, tick_clock, wait_clock):
        drain_inst = self.nc.sync.drain()
        wait_clock.add_sem_waits(
            drain_inst.ins, ScopedClock({None: tick_clock.global_clock}))
        si = drain_inst.ins.sync_info
        waits = list(si.on_wait or []) if si else []
        if len(waits) > 1:
            si.on_wait = waits[:1]
            for w in waits[1:]:
                d2 = self.nc.sync.drain()
                d2.ins.sync_info = mybir.SyncInfo(on_wait=[w], on_update=[])

        self.nc.all_engine_barrier()
        assert self.sems is not None
        popped = self.nc._tile_sem_poison_stack.pop()
        assert popped is self._sem_poison
        self.nc.clear_and_free_semaphores(list(self.sems.allocated().values()))
        self.nc.all_engine_barrier()

    tile.TileContext._drain_and_barrier = _drain_and_barrier
    tile.TileContext._drain_split_patched = True


_ENGINE_SEMS = ("PE", "Activation", "DVE", "Pool", "SP")


def _strip_redundant_waits(nc):
    """Walrus allows one sem wait per engine instruction.  Tile sometimes
    emits waits on an instruction's OWN engine semaphore whose producer is an
    earlier instruction in the same (in-order) engine stream — those are
    guaranteed satisfied and can be dropped.  DMA-completion sems (async) are
    never dropped."""
    for fn in nc.m.functions:
        for blk in fn.blocks:
            ins_list = list(blk.instructions)
            events = {}  # sem id -> list of (cum_count, idx, engine, type)
            for idx, ins in enumerate(ins_list):
                si = ins.sync_info
                if not si:
                    continue
                for u in (si.on_update or []):
                    lst = events.setdefault(u.id, [])
                    cum = (lst[-1][0] if lst else 0) + 1
                    lst.append((cum, idx, str(ins.engine), type(ins).__name__))
            for idx, ins in enumerate(ins_list):
                si = ins.sync_info
                if not si or not si.on_wait or len(si.on_wait) <= 1:
                    continue
                keep = []
                for w in si.on_wait:
                    name = (w.ant_name or "")
                    base = name.split("_")[0]
                    drop = False
                    if base in _ENGINE_SEMS and w.wait_mode == "sem-ge-imm":
                        prod = None
                        for cum, pidx, peng, ptype in events.get(w.id, []):
                            if cum >= w.wait_value:
                                prod = (pidx, peng, ptype)
                                break
                        if (prod is not None and prod[0] < idx
                                and prod[1] == str(ins.engine)
                                and prod[2] != "InstDMACopy"):
                            drop = True
                    if not drop:
                        keep.append(w)
                if len(keep) < len(si.on_wait):
                    si.on_wait = keep


def _build_program():
    import concourse.bass as bass
    import concourse.tile as tile
    from concourse import mybir
    _patch_drain_split(tile, mybir)

    f32 = mybir.dt.float32
    bf16 = mybir.dt.bfloat16
    f32r = mybir.dt.float32r if MM_MODE == "f32r" else bf16
    Alu = mybir.AluOpType
    Act = mybir.ActivationFunctionType

    nc = bass.Bass("TRN2", target_bir_lowering=False, debug=False)

    vb_d = nc.dram_tensor("vb", [BPC, 128, NKB, D], bf16,
                          kind="ExternalInput").ap()
    ad_d = nc.dram_tensor("adiag", [BPC, 128, K_LEN], bf16,
                          kind="ExternalInput").ap()
    scm_d = nc.dram_tensor("scm", [BPC, NKB + 1, D + K_LEN], f32r,
                           kind="ExternalInput").ap()
    wv_d = nc.dram_tensor("w_v", [128, 4, D], bf16, kind="ExternalInput").ap()
    out_d = nc.dram_tensor("out", [BPC, 128, NKB, D], bf16,
                           kind="ExternalOutput").ap()

    from contextlib import ExitStack
    from concourse.tile_rust import add_dep_helper
    with tile.TileContext(nc) as tc, ExitStack() as ctx:
        consts = ctx.enter_context(tc.tile_pool(name="consts", bufs=1))
        io_pool = ctx.enter_context(tc.tile_pool(name="io", bufs=2))
        vpool = ctx.enter_context(tc.tile_pool(name="v", bufs=2))
        pt_pool = ctx.enter_context(tc.tile_pool(name="pt", bufs=2))
        # 8 bufs x 4 tags = 32 x-tiles: every x allocation in the kernel is
        # distinct, so stt never carries a WAR wait for x reuse.
        xpool = ctx.enter_context(tc.tile_pool(name="x", bufs=8))
        sqpool = ctx.enter_context(tc.tile_pool(name="sq", bufs=2))
        stats = ctx.enter_context(tc.tile_pool(name="st", bufs=3))
        ypool = ctx.enter_context(tc.tile_pool(name="y", bufs=2))
        tpool = ctx.enter_context(tc.tile_pool(name="tp", bufs=1))
        pp_ps = ctx.enter_context(tc.tile_pool(name="pp", bufs=4, space="PSUM"))
        pa_ps = ctx.enter_context(tc.tile_pool(name="pa", bufs=3, space="PSUM"))
        dps = ctx.enter_context(tc.tile_pool(name="dps", bufs=1, space="PSUM"))
        dummy = dps.tile([1, 128], f32, tag="dummy")

        # Walrus allows only ONE semaphore wait on most engine-instruction
        # structs.  A "touch" is a tiny real op with a data dep on a producer:
        # it observes that producer's semaphore lane so the heavy op after it
        # (pinned via add_dep_helper) needs fewer waits of its own.
        _tn = [0]

        def pe_touch(ap11):
            if ap11.dtype == mybir.dt.float32r:
                ap11 = ap11.bitcast(f32)
            return nc.tensor.matmul(dummy[:1, :1], lhsT=ap11, rhs=ap11,
                                    start=True, stop=True,
                                    skip_group_check=True)

        def scratch():
            _tn[0] += 1
            t = tpool.tile([1, 1], f32, tag=f"t{_tn[0]}")
            return t

        def dve_touch(ap11):
            return nc.vector.tensor_copy(scratch()[:], ap11)

        def act_touch(ap11):
            return nc.scalar.copy(scratch()[:], ap11)

        def gp_touch(ap11):
            return nc.gpsimd.tensor_copy(scratch()[:], ap11)

        def order(op, pre_list):
            for t in pre_list:
                add_dep_helper(op.ins, t.ins, sync=False,
                               reason="ordered after wait-carrier")

        # wv on the scalar ring so the sync ring starts with batch 0's
        # adiag/scm (prologue critical path).  Never use the gpsimd ring for
        # bulk data: it is SWDGE (software descriptor generation, ~1us per
        # 4KB descriptor) and its stragglers slow the shared SDMA engines.
        wv_all = consts.tile([128, 4, D], bf16, tag="wv")
        nc.scalar.dma_start(wv_all[:], wv_d)
        wv_t = [wv_all[:, dc, :] for dc in range(4)]
        eps_t = consts.tile([128, 1], f32, tag="eps")
        nc.vector.memset(eps_t[:], LN_EPS)
        # PE prewarm: the first ~10us are DMA-bound and the PE would sit
        # idle and clock-gate cold (HAM K=4/8).  A stream of dummy matmuls
        # keeps the activity window busy so the real matmuls start at 2.4GHz.
        dumsrc = consts.tile([1, 128], bf16, tag="dumsrc")
        nc.vector.memset(dumsrc[:], 0)
        for _ in range(PREWARM_MMS):
            nc.tensor.matmul(dummy[:1, :128], lhsT=dumsrc[:1, :1],
                             rhs=dumsrc[:], start=True, stop=True,
                             skip_group_check=True)

        pt_hist = []    # pt tiles, pp allocation order
        x_hist = []     # x tiles, pa allocation order
        sq_hist = []    # square scratch tiles
        pending = [None]
        for b in range(BPC):
            # ---- loads: small tensors first so the first diag matmuls start
            # early.  One tile per V q-chunk: tile-granular dep tracking means
            # a single big tile would make chunk 0's readers wait for ALL
            # chunk DMAs.  Split across two HWDGE rings (sync + gpsimd) for
            # descriptor-issue overlap. ----
            # scm spans only 17 partitions (5 SDMA engines, fat 10KB
            # descriptors) — keep it off the sync ring so it can't stall the
            # 128-partition V loads behind it.
            scm = io_pool.tile([NKB + 1, D + K_LEN], f32r, tag="scm")
            nc.scalar.dma_start(scm[:], scm_d[b])
            v_q = []
            ad_q = []
            for jq in range(NQC):
                aq = io_pool.tile([128, 512], bf16, tag=f"ad{jq}")
                nc.sync.dma_start(aq[:], ad_d[b][:, 512 * jq:512 * (jq + 1)])
                vq = vpool.tile([128, 4, D], bf16, tag=f"v{jq}")
                nc.sync.dma_start(vq[:], vb_d[b][:, 4 * jq:4 * (jq + 1), :])
                ad_q.append(aq)
                v_q.append(vq)
            vb_t = [v_q[j // 4][:, j % 4, :] for j in range(NKB)]
            y_all = ypool.tile([128, NKB, D], bf16, tag="y")
            pe_pre = []
            dve_pre = []

            def emit_pt(jq, v_q, vb_t, ad_q, scm, pe_pre_l):
                t_ad = pe_touch(ad_q[jq][:1, :1])
                t_vb = pe_touch(v_q[jq][:1, 0, :1])
                pts = []
                pps = []
                for dc in range(4):
                    pre = pe_pre_l + ([t_ad, t_vb] if dc == 0 else [])
                    pe_pre_l = []
                    pp = pp_ps.tile([128, 512], f32, tag="pp")
                    first = None
                    for jj in range(4):
                        j = 4 * jq + jj
                        m = nc.tensor.matmul(
                            pp[:, 128 * jj:128 * (jj + 1)],
                            lhsT=vb_t[j][:, 128 * dc:128 * (dc + 1)],
                            rhs=ad_q[jq][:, 128 * jj:128 * (jj + 1)],
                            start=(jj == 0), stop=False, skip_group_check=True,
                        )
                        if first is None:
                            first = m
                            order(m, pre)
                    pps.append(pp)
                for dc in range(4):
                    nc.tensor.matmul(
                        pps[dc][:, :],
                        lhsT=scm[:, 128 * dc:128 * (dc + 1)],
                        rhs=scm[:, D + 512 * jq:D + 512 * (jq + 1)],
                        start=False, stop=True, skip_group_check=True,
                    )
                for dc in range(4):
                    pt = pt_pool.tile([128, 512], bf16, tag=f"pt{dc}")
                    if dc in EVAC_DVE:
                        i_evac = nc.vector.tensor_copy(pt[:], pps[dc][:])
                    else:
                        i_evac = nc.scalar.copy(pt[:], pps[dc][:])
                    pt_hist.append(pt)
                    pts.append(pt)
                return pts

            def emit_out(bb, jq, pts, v_q_b, vb_t_b, y_all_b, dve_pre_l):
                t_pts = pe_touch(pts[3][:1, :1])
                dve_pre_l = dve_pre_l + [dve_touch(v_q_b[jq][:1, 0, :1])]
                # tail mode (last batch, last two chunks): spread squares and
                # affines over both DVE and ACT; final chunk runs as two
                # half-groups so its stats/affine/DMA chain pipelines.
                tail = (bb == BPC - 1 and jq >= NQC - 2)
                halves = 2 if (bb == BPC - 1 and jq == NQC - 1) else 1
                njj = 4 // halves
                sx = stats.tile([128, 4], f32, tag="sx")
                sx2 = stats.tile([128, 4], f32, tag="sx2")
                lo = 4 * jq
                for h in range(halves):
                    x_t = []
                    for jj2 in range(njj):
                        jj = h * njj + jj2
                        j = 4 * jq + jj
                        pre = [t_pts] if jj == 0 else []
                        pa = pa_ps.tile([128, 512], f32, tag="pa")
                        first = None
                        for dc in range(4):
                            m = nc.tensor.matmul(
                                pa[:, :],
                                lhsT=pts[dc][:, 128 * jj:128 * (jj + 1)],
                                rhs=wv_t[dc][:],
                                start=(dc == 0), stop=(dc == 3),
                            )
                            if first is None:
                                first = m
                                order(m, pre)
                        x = xpool.tile([128, 512], f32, tag=f"x{jj}")
                        i_stt = nc.vector.scalar_tensor_tensor(
                            out=x[:], in0=pa[:], scalar=1.0,
                            in1=vb_t_b[j],
                            op0=Alu.mult, op1=Alu.add,
                            accum_out=sx[:, jj:jj + 1],
                        )
                        order(i_stt, dve_pre_l)
                        dve_pre_l = []
                        sq = sqpool.tile([128, 512], f32, tag="sq")
                        if tail and jj % 2 == 1:
                            nc.vector.scalar_tensor_tensor(
                                out=sq[:], in0=x[:], scalar=1.0, in1=x[:],
                                op0=Alu.mult, op1=Alu.mult,
                                accum_out=sx2[:, jj:jj + 1])
                        else:
                            nc.scalar.activation(
                                sq[:], x[:], Act.Square,
                                accum_out=sx2[:, jj:jj + 1])
                        sq_hist.append(sq)
                        x_t.append(x)
                        x_hist.append(x)

                    # var = sx2/D - (sx/D)^2 = (D*sx2 - sx^2)/D^2
                    sl = slice(h * njj, h * njj + njj)
                    mu_t = stats.tile([128, 4], f32, tag=f"mu{h}")
                    mu = mu_t[:, :njj]
                    nc.vector.tensor_scalar_mul(mu, sx[:, sl], 1.0 / D)
                    t1_t = stats.tile([128, 4], f32, tag=f"t1{h}")
                    t1 = t1_t[:, :njj]
                    nc.vector.tensor_mul(t1, sx[:, sl], sx[:, sl])
                    va_t = stats.tile([128, 4], f32, tag=f"va{h}")
                    va = va_t[:, :njj]
                    nc.vector.scalar_tensor_tensor(
                        out=va, in0=sx2[:, sl], scalar=float(D), in1=t1,
                        op0=Alu.mult, op1=Alu.subtract)
                    sd_t = stats.tile([128, 4], f32, tag=f"sd{h}")
                    sd = sd_t[:, :njj]
                    nc.scalar.activation(sd, va, Act.Sqrt,
                                         bias=eps_t[:], scale=1.0 / (D * D))
                    r_t = stats.tile([128, 4], f32, tag=f"r{h}")
                    r = r_t[:, :njj]
                    nc.vector.reciprocal(r, sd)
                    if tail:
                        nmur_t = stats.tile([128, 4], f32, tag=f"nm{h}")
                        nmur = nmur_t[:, :njj]
                        nc.vector.scalar_tensor_tensor(
                            out=nmur, in0=mu, scalar=-1.0, in1=r,
                            op0=Alu.mult, op1=Alu.mult)
                    for jj2 in range(njj):
                        jj = h * njj + jj2
                        j = 4 * jq + jj
                        if tail and jj % 2 == 1:
                            nc.scalar.activation(
                                y_all_b[:, j, :], x_t[jj2][:], Act.Identity,
                                bias=nmur[:, jj2:jj2 + 1],
                                scale=r[:, jj2:jj2 + 1])
                        else:
                            nc.vector.tensor_scalar(
                                out=y_all_b[:, j, :], in0=x_t[jj2][:],
                                scalar1=mu[:, jj2:jj2 + 1],
                                scalar2=r[:, jj2:jj2 + 1],
                                op0=Alu.subtract, op1=Alu.mult,
                            )
                    if halves == 2:
                        # touch the DVE-written (even) block; the ACT-written
                        # odd block is covered by ACT program order
                        t_y = act_touch(y_all_b[:1, lo + 2 * h, :1])
                        i_dma = nc.scalar.dma_start(
                            out_d[bb][:, lo + 2 * h:lo + 2 * h + 2, :],
                            y_all_b[:, lo + 2 * h:lo + 2 * h + 2, :])
                        order(i_dma, [t_y])
                if halves == 1:
                    t_y = act_touch(y_all_b[:1, lo + (3 if not tail else 2),
                                            :1])
                    i_dma = nc.scalar.dma_start(
                        out_d[bb][:, lo:lo + 4, :], y_all_b[:, lo:lo + 4, :])
                    order(i_dma, [t_y])

            # software pipeline: build P^T(jq) before finishing chunk jq-1,
            # so the PE fills evac waits with the next chunk's diag matmuls
            for jq in range(NQC):
                pts = emit_pt(jq, v_q, vb_t, ad_q, scm, pe_pre)
                pe_pre = []
                if pending[0] is not None:
                    emit_out(*pending[0])
                if b == 0 and jq < NQC - 1:
                    for _ in range(PREWARM_GAP):
                        nc.tensor.matmul(dummy[:1, :128], lhsT=dumsrc[:1, :1],
                                         rhs=dumsrc[:], start=True, stop=True,
                                         skip_group_check=True)
                pending[0] = (b, jq, pts, v_q, vb_t, y_all, dve_pre)
                dve_pre = []

        emit_out(*pending[0])

    _strip_redundant_waits(nc)
    return nc


def _get_program():
    if "nc" not in _COMPILED:
        _COMPILED["nc"] = _build_program()
    return _COMPILED["nc"]


def make_in_maps(pre, W_v):
    import ml_dtypes
    wv_in = np.ascontiguousarray(
        W_v.astype(np.float32).reshape(4, 128, D).transpose(1, 0, 2)
    ).astype(ml_dtypes.bfloat16)
    in_maps = []
    for c in range(N_CORES):
        sl = slice(c * BPC, (c + 1) * BPC)
        in_maps.append({
            "vb": np.ascontiguousarray(pre["vb"][sl]),
            "adiag": np.ascontiguousarray(pre["adiag"][sl]),
            "scm": np.ascontiguousarray(pre["scm"][sl]),
            "w_v": wv_in,
        })
    return in_maps


def assemble_out(results):
    # results[c]["out"]: (BPC, 128, NKB, D) bf16 -> (B, K_LEN, D) f32
    out = np.concatenate([results[c]["out"] for c in range(N_CORES)], axis=0)
    out = out.astype(np.float32).transpose(0, 2, 1, 3).reshape(B, K_LEN, D)
    return np.ascontiguousarray(out)


def kernel(Q, K, V, mask, W_q, W_k, W_v, ln_gamma, ln_beta):
    from concourse import bass_utils

    Q = np.asarray(Q); K = np.asarray(K); V = np.asarray(V)
    mask = np.asarray(mask)
    W_q = np.asarray(W_q); W_k = np.asarray(W_k); W_v = np.asarray(W_v)

    pre = _host_prep(Q, K, V, mask, W_q, W_k, W_v)
    in_maps = make_in_maps(pre, W_v)

    nc = _get_program()
    res = bass_utils.run_bass_kernel_spmd(nc, in_maps, list(range(N_CORES)))
    out = assemble_out(res.results)

    if not (np.all(ln_gamma == 1.0) and np.all(ln_beta == 0.0)):
        out = out * np.asarray(ln_gamma)[None, None, :] + \
            np.asarray(ln_beta)[None, None, :]
    return out.astype(np.float32)
